# revision 31
# baseline (speedup 1.0000x reference)
"""Masked (causal) multi-head self-attention on 8 Trainium2 NeuronCores.

Problem: x[2,2048,1024], Wq/Wk/Wv[1024,1024], biases[1024]; H=16 heads, hd=64.
Returns (out, attn, K, V) matching the jax reference:
    Q = x@Wq+bq; K = x@Wk+bk; V = x@Wv+bv
    attn = softmax(mask(QK^T/sqrt(hd)));  out = attn@V

Sharding: batch x head-group. 8 cores = 2 batch samples x 4 head-groups
(4 heads each). Wq/Wk/Wv are split column-wise per head group, so every
core runs an identical SPMD program on its slice with zero cross-core
communication.

Per-core program (all fp32):
  - x^T built via PE transposes (needed since PE contracts the partition dim)
  - Q^T,K^T,V^T = W^T x^T (transposed projections; per-partition bias folds
    into the PSUM->SBUF eviction on the scalar engine)
  - scores^T tiles [128k x 512q] = K_h Q_h^T via single matmuls (contraction=64)
  - exp on scalar engine (scale=1/8 folded in); causal mask via affine_select
    (fill=0 where q < k) -- only diagonal-crossing tiles need it
  - out^T accumulated as V_aug^T @ attn^T where V_aug carries a ones column:
    row 64 of out^T is the softmax denominator for free
  - attn tiles transposed back to [q,k] on PE; the PSUM eviction doubles as
    the softmax normalization (tensor_scalar x 1/sum); upper-triangle blocks
    are never written (outputs are zero-initialized by the runtime)
"""

import os
import sys

for _p in ("/opt/trn_rl_repo", "/root/.axon_site/_ro/trn_rl_repo"):
    if os.path.isdir(_p) and _p not in sys.path:
        sys.path.insert(0, _p)

import numpy as np

import concourse.bass as bass
import concourse.mybir as mybir
import concourse.tile as tile
from concourse import bacc
from concourse.bass_utils import run_bass_kernel_spmd
from concourse.masks import make_identity

F32 = mybir.dt.float32
F32R = mybir.dt.float32r

def _r(ap):
    return ap.bitcast(F32R)

N_CORES = 8
B, S, D = 2, 2048, 1024
H, HD = 16, 64
NHC = 4               # heads per core
DHC = NHC * HD        # 256 output columns per core
P = 128
NS = S // P           # 16 s-blocks
NKIN = D // P         # 8 contraction chunks for projections
NQS = 4               # q strips of 512
QW = 512              # strip width

_CACHED = {}


def _build_program():
    nc = bacc.Bacc("TRN2", target_bir_lowering=False, debug=False,
                   num_devices=N_CORES)

    xin = nc.declare_dram_parameter("xin", [S, D], F32R, isOutput=False)
    wq = nc.declare_dram_parameter("wq", [D, DHC], F32R, isOutput=False)
    wk = nc.declare_dram_parameter("wk", [D, DHC], F32R, isOutput=False)
    wv = nc.declare_dram_parameter("wv", [D, DHC], F32R, isOutput=False)
    bq = nc.declare_dram_parameter("bq", [DHC], F32, isOutput=False)
    bk = nc.declare_dram_parameter("bk", [DHC], F32, isOutput=False)
    bv = nc.declare_dram_parameter("bv", [DHC], F32, isOutput=False)

    outp = nc.declare_dram_parameter("outp", [S, DHC], F32, isOutput=True)
    attnp = nc.declare_dram_parameter("attnp", [NHC, S, S], F32, isOutput=True)
    kp = nc.declare_dram_parameter("kp", [S, DHC], F32R, isOutput=True)
    vp = nc.declare_dram_parameter("vp", [S, DHC], F32R, isOutput=True)

    Exp = mybir.ActivationFunctionType.Exp

    from contextlib import ExitStack
    with tile.TileContext(nc) as tc, ExitStack() as es:
        const = es.enter_context(tc.tile_pool(name="const", bufs=1))
        identf = const.tile([P, P], F32)
        make_identity(nc, identf)
        ident = const.tile([P, P], F32R)
        nc.scalar.copy(out=ident, in_=identf)

        # persistent per-core tensors
        persist = es.enter_context(tc.tile_pool(name="persist", bufs=1))
        QT = []
        KT = []
        for m in range(2):
            qtc = persist.tile([P, S], F32R, tag=f"QT{m}", name=f"QT{m}")
            ktc = persist.tile([P, S], F32R, tag=f"KT{m}", name=f"KT{m}")
            QT.append(qtc)
            KT.append(ktc)
        Vaug = persist.tile([P, NS, NHC * 66], F32R)  # V | ones | zero-pad per head
        outnat = persist.tile([P, NS, DHC], F32)
        bsb = persist.tile([P, 3, 2], F32)       # bq,bk,bv as per-partition cols

        # PSUM pools (8 banks total: 3 + 2 + 3)
        psmm = es.enter_context(tc.tile_pool(name="psmm", bufs=2, space="PSUM"))
        psout = es.enter_context(tc.tile_pool(name="psout", bufs=1, space="PSUM"))
        psstg = es.enter_context(tc.tile_pool(name="psstg", bufs=3, space="PSUM"))

        # ---------------- phase 1: x^T, projections, K/V layouts ----------
        with tc.tile_pool(name="ph1", bufs=1) as ph1, \
             tc.tile_pool(name="xload", bufs=3) as xload, \
             tc.tile_pool(name="kvstage", bufs=4) as kvstage:
            xT = ph1.tile([P, NKIN, S], F32R)    # [kin%128, kin//128, s]
            def xgroup(g):
                xt = xload.tile([P, D], F32R, tag="xt", name=f"xt{g}")
                nc.sync.dma_start(out=xt, in_=xin[g * P:(g + 1) * P, :])
                for c0 in range(0, NKIN, 4):
                    pst = psstg.tile([P, 512], F32, tag="stg",
                                     name=f"pstx{g}_{c0}")
                    for cc in range(4):
                        nc.tensor.transpose(
                            _r(pst[:, cc * P:(cc + 1) * P]),
                            _r(xt[:, (c0 + cc) * P:(c0 + cc + 1) * P]),
                            _r(ident))
                    nc.scalar.copy(out=xT[:, c0:c0 + 4, g * P:(g + 1) * P],
                                   in_=_r(pst))

            for g in range(3):
                xgroup(g)
            wsb = {}
            for name, h in (("wq", wq), ("wk", wk), ("wv", wv)):
                t = ph1.tile([P, NKIN, DHC], F32R, tag=f"wsb_{name}",
                             name=f"wsb_{name}")
                nc.sync.dma_start(out=t, in_=h[:].rearrange(
                    "(c p) n -> p c n", p=P))
                wsb[name] = t
            VT = []
            for m in range(2):
                vtc = ph1.tile([P, S], F32R, tag=f"VT{m}", name=f"VT{m}")
                VT.append(vtc)
            for bi, h in ((0, bq), (1, bk), (2, bv)):
                nc.sync.dma_start(out=bsb[:, bi, :], in_=h[:].rearrange(
                    "(c p) -> p c", p=P))
            for g in range(3, NS):
                xgroup(g)

            # projections: dest^T[do, s] += W_chunk^T @ x^T_chunk
            for (wname, bi, dest) in (("wv", 2, VT), ("wk", 1, KT),
                                      ("wq", 0, QT)):
                w = wsb[wname]
                for mo in range(2):
                    for st0 in range(0, NQS, 2):
                        ps = psmm.tile([P, 2, QW], F32, tag="mm")
                        for sub in range(2):
                            st = st0 + sub
                            for c in range(NKIN):
                                nc.tensor.matmul(
                                    ps[:, sub, :],
                                    lhsT=_r(w[:, c, mo * P:(mo + 1) * P]),
                                    rhs=_r(xT[:, c, st * QW:(st + 1) * QW]),
                                    start=(c == 0), stop=(c == NKIN - 1))
                        nc.scalar.add(
                            out=dest[mo][:, st0 * QW:(st0 + 2) * QW].rearrange(
                                "p (a b) -> p a b", a=2),
                            in_=ps, add=bsb[:, bi, mo:mo + 1])

            # K natural (for HBM) and V_aug (for out-matmul + HBM V)
            for h in range(NHC):
                nc.vector.memset(
                    Vaug[:, :, h * 66 + 64:h * 66 + 65].bitcast(F32), 1.0)
                nc.vector.memset(
                    Vaug[:, :, h * 66 + 65:h * 66 + 66].bitcast(F32), 0.0)
            for g in range(NS):
                psk = psstg.tile([P, 512], F32, tag="stg")
                for mo in range(2):
                    nc.tensor.transpose(_r(psk[:, mo * P:(mo + 1) * P]),
                                        _r(KT[mo][:, g * P:(g + 1) * P]),
                                        _r(ident))
                knat = kvstage.tile([P, DHC], F32R, tag="kv")
                nc.vector.tensor_copy(knat, _r(psk[:, :DHC]))
                nc.sync.dma_start(out=kp[g * P:(g + 1) * P, :], in_=knat)

                psv = psstg.tile([P, 512], F32, tag="stg")
                for mo in range(2):
                    nc.tensor.transpose(_r(psv[:, mo * P:(mo + 1) * P]),
                                        _r(VT[mo][:, g * P:(g + 1) * P]),
                                        _r(ident))
                nc.vector.tensor_copy(
                    Vaug[:, g, :].rearrange("p (h c) -> p h c", c=66)
                    [:, :, :HD],
                    _r(psv[:, :DHC].rearrange("p (h c) -> p h c", c=HD)))
                nc.sync.dma_start(
                    out=vp[g * P:(g + 1) * P, :],
                    in_=Vaug[:, g, :].rearrange("p (h c) -> p h c", c=66)
                    [:, :, :HD])

        # ---------------- phase 2: attention ------------------------------
        with tc.tile_pool(name="atts", bufs=16) as attp, \
             tc.tile_pool(name="rows", bufs=4) as rowp, \
             tc.tile_pool(name="outts", bufs=2) as outtp, \
             tc.tile_pool(name="recips", bufs=8) as recp:
            for h in range(NHC):
                ch, r0 = h // 2, 64 * (h % 2)
                qTh = QT[ch][r0:r0 + 64, :]
                kTh = KT[ch][r0:r0 + 64, :]
                for i in (3, 2, 1, 0):
                    nj = 4 * i + 4
                    atts = []
                    for j0 in range(0, nj, 2):
                        ps = psmm.tile([P, 2, QW], F32, tag="mm")
                        for sub in range(2):
                            nc.tensor.matmul(
                                ps[:, sub, :],
                                lhsT=_r(kTh[:, (j0 + sub) * P:(j0 + sub + 1) * P]),
                                rhs=_r(qTh[:, i * QW:(i + 1) * QW]),
                                start=True, stop=True)
                        at = attp.tile([P, 2, QW], F32R, tag="att")
                        nc.scalar.activation(out=at, in_=ps, func=Exp,
                                             scale=0.125)
                        atts.append(at[:, 0, :])
                        atts.append(at[:, 1, :])
                    # causal mask on diagonal-crossing tiles:
                    # keep where q >= k  <=>  qq - r + (512i - 128j) >= 0
                    for j in range(4 * i, nj):
                        nc.gpsimd.affine_select(
                            out=atts[j], in_=atts[j], pattern=[[1, QW]],
                            compare_op=mybir.AluOpType.is_ge, fill=0.0,
                            base=QW * i - P * j, channel_multiplier=-1)
                    # out^T (+ softmax sums in row 64) accumulation
                    pout = psout.tile([P, QW], F32, tag="po")
                    for j in range(nj):
                        nc.tensor.matmul(
                            pout[:66, :], lhsT=_r(Vaug[:, j, h * 66:(h + 1) * 66]),
                            rhs=_r(atts[j]), start=(j == 0), stop=(j == nj - 1))
                    outT = outtp.tile([66, QW], F32R, tag="outT")
                    nc.scalar.copy(out=outT, in_=pout[:66, :])
                    recs = []
                    for qq in range(4):
                        g = 4 * i + qq
                        pt = psstg.tile([P, 512], F32, tag="stg")
                        nc.tensor.transpose(_r(pt[:, :66]),
                                            _r(outT[:, qq * P:(qq + 1) * P]),
                                            _r(ident[:66, :66]))
                        rec = recp.tile([P, 1], F32, tag="rec")
                        nc.vector.reciprocal(rec, pt[:, 64:65])
                        nc.vector.tensor_scalar_mul(
                            outnat[:, g, h * HD:(h + 1) * HD],
                            pt[:, :HD], rec)
                        recs.append(rec)
                    # attn rows: transpose attnT blocks, normalize on evict
                    for qq in range(4):
                        g = 4 * i + qq
                        row = rowp.tile([P, S], F32, tag="row")
                        for j0 in range(0, g + 1, 4):
                            jc = min(4, g + 1 - j0)
                            pst = psstg.tile([P, 512], F32, tag="stg")
                            for jj in range(jc):
                                nc.tensor.transpose(
                                    _r(pst[:, jj * P:(jj + 1) * P]),
                                    _r(atts[j0 + jj][:, qq * P:(qq + 1) * P]),
                                    _r(ident))
                            nc.vector.tensor_scalar_mul(
                                row[:, j0 * P:(j0 + jc) * P],
                                pst[:, :jc * P], recs[qq])
                        nc.sync.dma_start(
                            out=attnp[h, g * P:(g + 1) * P, :(g + 1) * P],
                            in_=row[:, :(g + 1) * P])
                # per-head slice of the final out DMA (shrinks the tail)
                nc.sync.dma_start(
                    out=outp[:, h * HD:(h + 1) * HD].rearrange(
                        "(g p) n -> p g n", p=P),
                    in_=outnat[:, :, h * HD:(h + 1) * HD])


    nc.compile()
    return nc


def _get_nc():
    if "nc" not in _CACHED:
        _CACHED["nc"] = _build_program()
    return _CACHED["nc"]


def _run(x, Wq, bq, Wk, bk, Wv, bv, trace=False):
    nc = _get_nc()
    x = np.ascontiguousarray(x, dtype=np.float32)
    in_maps = []
    for c in range(N_CORES):
        b, hg = divmod(c, N_CORES // B)
        cs = slice(hg * DHC, (hg + 1) * DHC)
        in_maps.append({
            "xin": x[b],
            "wq": np.ascontiguousarray(Wq[:, cs], dtype=np.float32),
            "wk": np.ascontiguousarray(Wk[:, cs], dtype=np.float32),
            "wv": np.ascontiguousarray(Wv[:, cs], dtype=np.float32),
            "bq": np.ascontiguousarray(bq[cs], dtype=np.float32),
            "bk": np.ascontiguousarray(bk[cs], dtype=np.float32),
            "bv": np.ascontiguousarray(bv[cs], dtype=np.float32),
        })
    res = run_bass_kernel_spmd(nc, in_maps, list(range(N_CORES)),
                               trace=trace)
    out = np.empty((B, S, D), np.float32)
    attn = np.empty((B, H, S, S), np.float32)
    K = np.empty((B, S, D), np.float32)
    V = np.empty((B, S, D), np.float32)
    for c in range(N_CORES):
        b, hg = divmod(c, N_CORES // B)
        cs = slice(hg * DHC, (hg + 1) * DHC)
        r = res.results[c]
        out[b][:, cs] = r["outp"]
        attn[b, hg * NHC:(hg + 1) * NHC] = r["attnp"]
        K[b][:, cs] = r["kp"]
        V[b][:, cs] = r["vp"]
    return (out, attn, K, V), res


def kernel(x, Wq, bq, Wk, bk, Wv, bv):
    outs, _ = _run(x, Wq, bq, Wk, bk, Wv, bv, trace=False)
    return outs


# revision 34
# speedup vs baseline: 1.0080x; 1.0080x over previous
"""Masked (causal) multi-head self-attention on 8 Trainium2 NeuronCores.

Problem: x[2,2048,1024], Wq/Wk/Wv[1024,1024], biases[1024]; H=16 heads, hd=64.
Returns (out, attn, K, V) matching the jax reference:
    Q = x@Wq+bq; K = x@Wk+bk; V = x@Wv+bv
    attn = softmax(mask(QK^T/sqrt(hd)));  out = attn@V

Sharding: batch x head-group. 8 cores = 2 batch samples x 4 head-groups
(4 heads each). Wq/Wk/Wv are split column-wise per head group, so every
core runs an identical SPMD program on its slice with zero cross-core
communication.

Per-core program (all fp32):
  - x^T built via PE transposes (needed since PE contracts the partition dim)
  - Q^T,K^T,V^T = W^T x^T (transposed projections; per-partition bias folds
    into the PSUM->SBUF eviction on the scalar engine)
  - scores^T tiles [128k x 512q] = K_h Q_h^T via single matmuls (contraction=64)
  - exp on scalar engine (scale=1/8 folded in); causal mask via affine_select
    (fill=0 where q < k) -- only diagonal-crossing tiles need it
  - out^T accumulated as V_aug^T @ attn^T where V_aug carries a ones column:
    row 64 of out^T is the softmax denominator for free
  - attn tiles transposed back to [q,k] on PE; the PSUM eviction doubles as
    the softmax normalization (tensor_scalar x 1/sum); upper-triangle blocks
    are never written (outputs are zero-initialized by the runtime)
"""

import os
import sys

for _p in ("/opt/trn_rl_repo", "/root/.axon_site/_ro/trn_rl_repo"):
    if os.path.isdir(_p) and _p not in sys.path:
        sys.path.insert(0, _p)

import numpy as np

import concourse.bass as bass
import concourse.mybir as mybir
import concourse.tile as tile
from concourse import bacc
from concourse.bass_utils import run_bass_kernel_spmd
from concourse.masks import make_identity

F32 = mybir.dt.float32
F32R = mybir.dt.float32r

def _r(ap):
    return ap.bitcast(F32R)

N_CORES = 8
B, S, D = 2, 2048, 1024
H, HD = 16, 64
NHC = 4               # heads per core
DHC = NHC * HD        # 256 output columns per core
P = 128
NS = S // P           # 16 s-blocks
NKIN = D // P         # 8 contraction chunks for projections
NQS = 4               # q strips of 512
QW = 512              # strip width

_CACHED = {}


def _build_program():
    nc = bacc.Bacc("TRN2", target_bir_lowering=False, debug=False,
                   num_devices=N_CORES)

    xin = nc.declare_dram_parameter("xin", [S, D], F32R, isOutput=False)
    wq = nc.declare_dram_parameter("wq", [D, DHC], F32R, isOutput=False)
    wk = nc.declare_dram_parameter("wk", [D, DHC], F32R, isOutput=False)
    wv = nc.declare_dram_parameter("wv", [D, DHC], F32R, isOutput=False)
    bq = nc.declare_dram_parameter("bq", [DHC], F32, isOutput=False)
    bk = nc.declare_dram_parameter("bk", [DHC], F32, isOutput=False)
    bv = nc.declare_dram_parameter("bv", [DHC], F32, isOutput=False)

    outp = nc.declare_dram_parameter("outp", [S, DHC], F32, isOutput=True)
    attnp = nc.declare_dram_parameter("attnp", [NHC, S, S], F32, isOutput=True)
    kp = nc.declare_dram_parameter("kp", [S, DHC], F32R, isOutput=True)
    vp = nc.declare_dram_parameter("vp", [S, DHC], F32R, isOutput=True)

    Exp = mybir.ActivationFunctionType.Exp

    from contextlib import ExitStack
    with tile.TileContext(nc) as tc, ExitStack() as es:
        const = es.enter_context(tc.tile_pool(name="const", bufs=1))
        identf = const.tile([P, P], F32)
        make_identity(nc, identf)
        ident = const.tile([P, P], F32R)
        nc.scalar.copy(out=ident, in_=identf)

        # persistent per-core tensors
        persist = es.enter_context(tc.tile_pool(name="persist", bufs=1))
        QT = []
        KT = []
        for m in range(2):
            qtc = persist.tile([P, S], F32R, tag=f"QT{m}", name=f"QT{m}")
            ktc = persist.tile([P, S], F32R, tag=f"KT{m}", name=f"KT{m}")
            QT.append(qtc)
            KT.append(ktc)
        Vaug = persist.tile([P, NS, NHC * 66], F32R)  # V | ones | zero-pad per head
        outnat = persist.tile([P, NS, DHC], F32)
        bsb = persist.tile([P, 3, 2], F32)       # bq,bk,bv as per-partition cols

        # PSUM pools (8 banks total: 3 + 2 + 3)
        psmm = es.enter_context(tc.tile_pool(name="psmm", bufs=2, space="PSUM"))
        psout = es.enter_context(tc.tile_pool(name="psout", bufs=1, space="PSUM"))
        psstg = es.enter_context(tc.tile_pool(name="psstg", bufs=3, space="PSUM"))

        # ---------------- phase 1: x^T, projections, K/V layouts ----------
        with tc.tile_pool(name="ph1", bufs=1) as ph1, \
             tc.tile_pool(name="xload", bufs=3) as xload, \
             tc.tile_pool(name="kvstage", bufs=4) as kvstage:
            xT = ph1.tile([P, NKIN, S], F32R)    # [kin%128, kin//128, s]
            def xgroup(g):
                xt = xload.tile([P, D], F32R, tag="xt", name=f"xt{g}")
                nc.sync.dma_start(out=xt, in_=xin[g * P:(g + 1) * P, :])
                for c0 in range(0, NKIN, 4):
                    pst = psstg.tile([P, 512], F32, tag="stg",
                                     name=f"pstx{g}_{c0}")
                    for cc in range(4):
                        nc.tensor.transpose(
                            _r(pst[:, cc * P:(cc + 1) * P]),
                            _r(xt[:, (c0 + cc) * P:(c0 + cc + 1) * P]),
                            _r(ident))
                    nc.scalar.copy(out=xT[:, c0:c0 + 4, g * P:(g + 1) * P],
                                   in_=_r(pst))

            for g in range(3):
                xgroup(g)
            wsb = {}
            for name, h in (("wq", wq), ("wk", wk), ("wv", wv)):
                t = ph1.tile([P, NKIN, DHC], F32R, tag=f"wsb_{name}",
                             name=f"wsb_{name}")
                nc.sync.dma_start(out=t, in_=h[:].rearrange(
                    "(c p) n -> p c n", p=P))
                wsb[name] = t
            VT = []
            for m in range(2):
                vtc = ph1.tile([P, S], F32R, tag=f"VT{m}", name=f"VT{m}")
                VT.append(vtc)
            for bi, h in ((0, bq), (1, bk), (2, bv)):
                nc.sync.dma_start(out=bsb[:, bi, :], in_=h[:].rearrange(
                    "(c p) -> p c", p=P))
            for g in range(3, NS):
                xgroup(g)

            # projections: dest^T[do, s] += W_chunk^T @ x^T_chunk
            for (wname, bi, dest) in (("wv", 2, VT), ("wk", 1, KT),
                                      ("wq", 0, QT)):
                w = wsb[wname]
                for mo in range(2):
                    for st0 in range(0, NQS, 2):
                        ps = psmm.tile([P, 2, QW], F32, tag="mm")
                        for sub in range(2):
                            st = st0 + sub
                            for c in range(NKIN):
                                nc.tensor.matmul(
                                    ps[:, sub, :],
                                    lhsT=_r(w[:, c, mo * P:(mo + 1) * P]),
                                    rhs=_r(xT[:, c, st * QW:(st + 1) * QW]),
                                    start=(c == 0), stop=(c == NKIN - 1))
                        nc.scalar.add(
                            out=dest[mo][:, st0 * QW:(st0 + 2) * QW].rearrange(
                                "p (a b) -> p a b", a=2),
                            in_=ps, add=bsb[:, bi, mo:mo + 1])

            # K natural (for HBM) and V_aug (for out-matmul + HBM V)
            for h in range(NHC):
                nc.vector.memset(
                    Vaug[:, :, h * 66 + 64:h * 66 + 65].bitcast(F32), 1.0)
                nc.vector.memset(
                    Vaug[:, :, h * 66 + 65:h * 66 + 66].bitcast(F32), 0.0)
            for g in range(NS):
                psk = psstg.tile([P, 512], F32, tag="stg")
                for mo in range(2):
                    nc.tensor.transpose(_r(psk[:, mo * P:(mo + 1) * P]),
                                        _r(KT[mo][:, g * P:(g + 1) * P]),
                                        _r(ident))
                knat = kvstage.tile([P, DHC], F32R, tag="kv")
                nc.vector.tensor_copy(knat, _r(psk[:, :DHC]))
                nc.sync.dma_start(out=kp[g * P:(g + 1) * P, :], in_=knat)

                psv = psstg.tile([P, 512], F32, tag="stg")
                for mo in range(2):
                    nc.tensor.transpose(_r(psv[:, mo * P:(mo + 1) * P]),
                                        _r(VT[mo][:, g * P:(g + 1) * P]),
                                        _r(ident))
                nc.vector.tensor_copy(
                    Vaug[:, g, :].rearrange("p (h c) -> p h c", c=66)
                    [:, :, :HD],
                    _r(psv[:, :DHC].rearrange("p (h c) -> p h c", c=HD)))
                nc.sync.dma_start(
                    out=vp[g * P:(g + 1) * P, :],
                    in_=Vaug[:, g, :].rearrange("p (h c) -> p h c", c=66)
                    [:, :, :HD])

        # ---------------- phase 2: attention ------------------------------
        with tc.tile_pool(name="atts", bufs=16) as attp, \
             tc.tile_pool(name="rows", bufs=5) as rowp, \
             tc.tile_pool(name="outts", bufs=2) as outtp, \
             tc.tile_pool(name="recips", bufs=8) as recp:
            for h in range(NHC):
                ch, r0 = h // 2, 64 * (h % 2)
                qTh = QT[ch][r0:r0 + 64, :]
                kTh = KT[ch][r0:r0 + 64, :]
                for i in (3, 2, 1, 0):
                    nj = 4 * i + 4
                    atts = []
                    for j0 in range(0, nj, 2):
                        ps = psmm.tile([P, 2, QW], F32, tag="mm")
                        for sub in range(2):
                            nc.tensor.matmul(
                                ps[:, sub, :],
                                lhsT=_r(kTh[:, (j0 + sub) * P:(j0 + sub + 1) * P]),
                                rhs=_r(qTh[:, i * QW:(i + 1) * QW]),
                                start=True, stop=True)
                        at = attp.tile([P, 2, QW], F32R, tag="att")
                        nc.scalar.activation(out=at, in_=ps, func=Exp,
                                             scale=0.125)
                        atts.append(at[:, 0, :])
                        atts.append(at[:, 1, :])
                    # causal mask on diagonal-crossing tiles:
                    # keep where q >= k  <=>  qq - r + (512i - 128j) >= 0
                    for j in range(4 * i, nj):
                        nc.gpsimd.affine_select(
                            out=atts[j], in_=atts[j], pattern=[[1, QW]],
                            compare_op=mybir.AluOpType.is_ge, fill=0.0,
                            base=QW * i - P * j, channel_multiplier=-1)
                    # out^T (+ softmax sums in row 64) accumulation
                    pout = psout.tile([P, QW], F32, tag="po")
                    for j in range(nj):
                        nc.tensor.matmul(
                            pout[:66, :], lhsT=_r(Vaug[:, j, h * 66:(h + 1) * 66]),
                            rhs=_r(atts[j]), start=(j == 0), stop=(j == nj - 1))
                    outT = outtp.tile([66, QW], F32R, tag="outT")
                    nc.scalar.copy(out=outT, in_=pout[:66, :])
                    recs = []
                    for qq in range(4):
                        g = 4 * i + qq
                        pt = psstg.tile([P, 512], F32, tag="stg")
                        nc.tensor.transpose(_r(pt[:, :66]),
                                            _r(outT[:, qq * P:(qq + 1) * P]),
                                            _r(ident[:66, :66]))
                        rec = recp.tile([P, 1], F32, tag="rec")
                        nc.vector.reciprocal(rec, pt[:, 64:65])
                        nc.vector.tensor_scalar_mul(
                            outnat[:, g, h * HD:(h + 1) * HD],
                            pt[:, :HD], rec)
                        recs.append(rec)
                    # attn rows: transpose attnT blocks, normalize on evict
                    for qq in range(4):
                        g = 4 * i + qq
                        row = rowp.tile([P, S], F32, tag="row")
                        for j0 in range(0, g + 1, 4):
                            jc = min(4, g + 1 - j0)
                            pst = psstg.tile([P, 512], F32, tag="stg")
                            for jj in range(jc):
                                nc.tensor.transpose(
                                    _r(pst[:, jj * P:(jj + 1) * P]),
                                    _r(atts[j0 + jj][:, qq * P:(qq + 1) * P]),
                                    _r(ident))
                            nc.vector.tensor_scalar_mul(
                                row[:, j0 * P:(j0 + jc) * P],
                                pst[:, :jc * P], recs[qq])
                        nc.sync.dma_start(
                            out=attnp[h, g * P:(g + 1) * P, :(g + 1) * P],
                            in_=row[:, :(g + 1) * P])
                # per-head slice of the final out DMA (shrinks the tail)
                nc.sync.dma_start(
                    out=outp[:, h * HD:(h + 1) * HD].rearrange(
                        "(g p) n -> p g n", p=P),
                    in_=outnat[:, :, h * HD:(h + 1) * HD])


    nc.compile()
    return nc


def _get_nc():
    if "nc" not in _CACHED:
        _CACHED["nc"] = _build_program()
    return _CACHED["nc"]


def _run(x, Wq, bq, Wk, bk, Wv, bv, trace=False):
    nc = _get_nc()
    x = np.ascontiguousarray(x, dtype=np.float32)
    in_maps = []
    for c in range(N_CORES):
        b, hg = divmod(c, N_CORES // B)
        cs = slice(hg * DHC, (hg + 1) * DHC)
        in_maps.append({
            "xin": x[b],
            "wq": np.ascontiguousarray(Wq[:, cs], dtype=np.float32),
            "wk": np.ascontiguousarray(Wk[:, cs], dtype=np.float32),
            "wv": np.ascontiguousarray(Wv[:, cs], dtype=np.float32),
            "bq": np.ascontiguousarray(bq[cs], dtype=np.float32),
            "bk": np.ascontiguousarray(bk[cs], dtype=np.float32),
            "bv": np.ascontiguousarray(bv[cs], dtype=np.float32),
        })
    res = run_bass_kernel_spmd(nc, in_maps, list(range(N_CORES)),
                               trace=trace)
    out = np.empty((B, S, D), np.float32)
    attn = np.empty((B, H, S, S), np.float32)
    K = np.empty((B, S, D), np.float32)
    V = np.empty((B, S, D), np.float32)
    for c in range(N_CORES):
        b, hg = divmod(c, N_CORES // B)
        cs = slice(hg * DHC, (hg + 1) * DHC)
        r = res.results[c]
        out[b][:, cs] = r["outp"]
        attn[b, hg * NHC:(hg + 1) * NHC] = r["attnp"]
        K[b][:, cs] = r["kp"]
        V[b][:, cs] = r["vp"]
    return (out, attn, K, V), res


def kernel(x, Wq, bq, Wk, bk, Wv, bv):
    outs, _ = _run(x, Wq, bq, Wk, bk, Wv, bv, trace=False)
    return outs


# revision 37
# speedup vs baseline: 1.0248x; 1.0168x over previous
"""Masked (causal) multi-head self-attention on 8 Trainium2 NeuronCores.

Problem: x[2,2048,1024], Wq/Wk/Wv[1024,1024], biases[1024]; H=16 heads, hd=64.
Returns (out, attn, K, V) matching the jax reference:
    Q = x@Wq+bq; K = x@Wk+bk; V = x@Wv+bv
    attn = softmax(mask(QK^T/sqrt(hd)));  out = attn@V

Sharding: batch x head-group. 8 cores = 2 batch samples x 4 head-groups
(4 heads each). Wq/Wk/Wv are split column-wise per head group, so every
core runs an identical SPMD program on its slice with zero cross-core
communication.

Per-core program (all fp32):
  - x^T built via PE transposes (needed since PE contracts the partition dim)
  - Q^T,K^T,V^T = W^T x^T (transposed projections; per-partition bias folds
    into the PSUM->SBUF eviction on the scalar engine)
  - scores^T tiles [128k x 512q] = K_h Q_h^T via single matmuls (contraction=64)
  - exp on scalar engine (scale=1/8 folded in); causal mask via affine_select
    (fill=0 where q < k) -- only diagonal-crossing tiles need it
  - out^T accumulated as V_aug^T @ attn^T where V_aug carries a ones column:
    row 64 of out^T is the softmax denominator for free
  - attn tiles transposed back to [q,k] on PE; the PSUM eviction doubles as
    the softmax normalization (tensor_scalar x 1/sum); upper-triangle blocks
    are never written (outputs are zero-initialized by the runtime)
"""

import os
import sys

for _p in ("/opt/trn_rl_repo", "/root/.axon_site/_ro/trn_rl_repo"):
    if os.path.isdir(_p) and _p not in sys.path:
        sys.path.insert(0, _p)

import numpy as np

import concourse.bass as bass
import concourse.mybir as mybir
import concourse.tile as tile
from concourse import bacc
from concourse.bass_utils import run_bass_kernel_spmd
from concourse.masks import make_identity

F32 = mybir.dt.float32
F32R = mybir.dt.float32r

def _r(ap):
    return ap.bitcast(F32R)

N_CORES = 8
B, S, D = 2, 2048, 1024
H, HD = 16, 64
NHC = 4               # heads per core
DHC = NHC * HD        # 256 output columns per core
P = 128
NS = S // P           # 16 s-blocks
NKIN = D // P         # 8 contraction chunks for projections
NQS = 4               # q strips of 512
QW = 512              # strip width

_CACHED = {}


def _build_program():
    nc = bacc.Bacc("TRN2", target_bir_lowering=False, debug=False,
                   num_devices=N_CORES)

    xin = nc.declare_dram_parameter("xin", [S, D], F32R, isOutput=False)
    wq = nc.declare_dram_parameter("wq", [D, DHC], F32R, isOutput=False)
    wk = nc.declare_dram_parameter("wk", [D, DHC], F32R, isOutput=False)
    wv = nc.declare_dram_parameter("wv", [D, DHC], F32R, isOutput=False)
    bq = nc.declare_dram_parameter("bq", [DHC], F32, isOutput=False)
    bk = nc.declare_dram_parameter("bk", [DHC], F32, isOutput=False)
    bv = nc.declare_dram_parameter("bv", [DHC], F32, isOutput=False)

    outp = nc.declare_dram_parameter("outp", [S, DHC], F32, isOutput=True)
    attnp = nc.declare_dram_parameter("attnp", [NHC, S, S], F32, isOutput=True)
    kp = nc.declare_dram_parameter("kp", [S, DHC], F32R, isOutput=True)
    vp = nc.declare_dram_parameter("vp", [S, DHC], F32R, isOutput=True)

    Exp = mybir.ActivationFunctionType.Exp

    from contextlib import ExitStack
    with tile.TileContext(nc) as tc, ExitStack() as es:
        const = es.enter_context(tc.tile_pool(name="const", bufs=1))
        identf = const.tile([P, P], F32)
        make_identity(nc, identf)
        ident = const.tile([P, P], F32R)
        nc.scalar.copy(out=ident, in_=identf)

        # persistent per-core tensors
        persist = es.enter_context(tc.tile_pool(name="persist", bufs=1))
        QT = []
        KT = []
        for m in range(2):
            qtc = persist.tile([P, S], F32R, tag=f"QT{m}", name=f"QT{m}")
            ktc = persist.tile([P, S], F32R, tag=f"KT{m}", name=f"KT{m}")
            QT.append(qtc)
            KT.append(ktc)
        Vaug = persist.tile([P, NS, NHC * 66], F32R)  # V | ones | zero-pad per head
        outnat = persist.tile([P, NS, DHC], F32)
        bsb = persist.tile([P, 3, 2], F32)       # bq,bk,bv as per-partition cols

        # PSUM pools (8 banks total: 3 + 2 + 3)
        psmm = es.enter_context(tc.tile_pool(name="psmm", bufs=2, space="PSUM"))
        psout = es.enter_context(tc.tile_pool(name="psout", bufs=1, space="PSUM"))
        psstg = es.enter_context(tc.tile_pool(name="psstg", bufs=3, space="PSUM"))

        # ---------------- phase 1: x^T, projections, K/V layouts ----------
        with tc.tile_pool(name="ph1", bufs=1) as ph1, \
             tc.tile_pool(name="xload", bufs=3) as xload, \
             tc.tile_pool(name="kvstage", bufs=4) as kvstage:
            xT = ph1.tile([P, NKIN, S], F32R)    # [kin%128, kin//128, s]
            def xgroup(g):
                xt = xload.tile([P, D], F32R, tag="xt", name=f"xt{g}")
                nc.sync.dma_start(out=xt, in_=xin[g * P:(g + 1) * P, :])
                for c0 in range(0, NKIN, 4):
                    pst = psstg.tile([P, 512], F32, tag="stg",
                                     name=f"pstx{g}_{c0}")
                    for cc in range(4):
                        nc.tensor.transpose(
                            _r(pst[:, cc * P:(cc + 1) * P]),
                            _r(xt[:, (c0 + cc) * P:(c0 + cc + 1) * P]),
                            _r(ident))
                    nc.scalar.copy(out=xT[:, c0:c0 + 4, g * P:(g + 1) * P],
                                   in_=_r(pst))

            for g in range(3):
                xgroup(g)
            wsb = {}
            for name, h in (("wq", wq), ("wk", wk), ("wv", wv)):
                t = ph1.tile([P, NKIN, DHC], F32R, tag=f"wsb_{name}",
                             name=f"wsb_{name}")
                nc.sync.dma_start(out=t, in_=h[:].rearrange(
                    "(c p) n -> p c n", p=P))
                wsb[name] = t
            VT = []
            for m in range(2):
                vtc = ph1.tile([P, S], F32R, tag=f"VT{m}", name=f"VT{m}")
                VT.append(vtc)
            for bi, h in ((0, bq), (1, bk), (2, bv)):
                nc.sync.dma_start(out=bsb[:, bi, :], in_=h[:].rearrange(
                    "(c p) -> p c", p=P))
            for g in range(3, NS):
                xgroup(g)

            # projections: dest^T[do, s] += W_chunk^T @ x^T_chunk
            for (wname, bi, dest) in (("wv", 2, VT), ("wk", 1, KT),
                                      ("wq", 0, QT)):
                w = wsb[wname]
                for mo in range(2):
                    for st0 in range(0, NQS, 2):
                        ps = psmm.tile([P, 2, QW], F32, tag="mm")
                        for sub in range(2):
                            st = st0 + sub
                            for c in range(NKIN):
                                nc.tensor.matmul(
                                    ps[:, sub, :],
                                    lhsT=_r(w[:, c, mo * P:(mo + 1) * P]),
                                    rhs=_r(xT[:, c, st * QW:(st + 1) * QW]),
                                    start=(c == 0), stop=(c == NKIN - 1))
                        nc.scalar.add(
                            out=dest[mo][:, st0 * QW:(st0 + 2) * QW].rearrange(
                                "p (a b) -> p a b", a=2),
                            in_=ps, add=bsb[:, bi, mo:mo + 1])

            # K natural (for HBM) and V_aug (for out-matmul + HBM V)
            for h in range(NHC):
                nc.vector.memset(
                    Vaug[:, :, h * 66 + 64:h * 66 + 65].bitcast(F32), 1.0)
                nc.vector.memset(
                    Vaug[:, :, h * 66 + 65:h * 66 + 66].bitcast(F32), 0.0)
            for g in range(NS):
                psk = psstg.tile([P, 512], F32, tag="stg")
                for mo in range(2):
                    nc.tensor.transpose(_r(psk[:, mo * P:(mo + 1) * P]),
                                        _r(KT[mo][:, g * P:(g + 1) * P]),
                                        _r(ident))
                knat = kvstage.tile([P, DHC], F32R, tag="kv")
                nc.vector.tensor_copy(knat, _r(psk[:, :DHC]))
                nc.sync.dma_start(out=kp[g * P:(g + 1) * P, :], in_=knat)

                psv = psstg.tile([P, 512], F32, tag="stg")
                for mo in range(2):
                    nc.tensor.transpose(_r(psv[:, mo * P:(mo + 1) * P]),
                                        _r(VT[mo][:, g * P:(g + 1) * P]),
                                        _r(ident))
                nc.vector.tensor_copy(
                    Vaug[:, g, :].rearrange("p (h c) -> p h c", c=66)
                    [:, :, :HD],
                    _r(psv[:, :DHC].rearrange("p (h c) -> p h c", c=HD)))
                nc.sync.dma_start(
                    out=vp[g * P:(g + 1) * P, :],
                    in_=Vaug[:, g, :].rearrange("p (h c) -> p h c", c=66)
                    [:, :, :HD])

        # ---------------- phase 2: attention ------------------------------
        with tc.tile_pool(name="atts", bufs=16) as attp, \
             tc.tile_pool(name="rows", bufs=6) as rowp, \
             tc.tile_pool(name="outts", bufs=2) as outtp, \
             tc.tile_pool(name="recips", bufs=8) as recp:
            for h in range(NHC):
                ch, r0 = h // 2, 64 * (h % 2)
                qTh = QT[ch][r0:r0 + 64, :]
                kTh = KT[ch][r0:r0 + 64, :]
                for i in (3, 2, 1, 0):
                    nj = 4 * i + 4
                    atts = []
                    for j0 in range(0, nj, 2):
                        ps = psmm.tile([P, 2, QW], F32, tag="mm")
                        for sub in range(2):
                            nc.tensor.matmul(
                                ps[:, sub, :],
                                lhsT=_r(kTh[:, (j0 + sub) * P:(j0 + sub + 1) * P]),
                                rhs=_r(qTh[:, i * QW:(i + 1) * QW]),
                                start=True, stop=True)
                        at = attp.tile([P, 2, QW], F32R, tag="att")
                        nc.scalar.activation(out=at, in_=ps, func=Exp,
                                             scale=0.125)
                        atts.append(at[:, 0, :])
                        atts.append(at[:, 1, :])
                    # causal mask on diagonal-crossing tiles:
                    # keep where q >= k  <=>  qq - r + (512i - 128j) >= 0
                    for j in range(4 * i, nj):
                        nc.gpsimd.affine_select(
                            out=atts[j], in_=atts[j], pattern=[[1, QW]],
                            compare_op=mybir.AluOpType.is_ge, fill=0.0,
                            base=QW * i - P * j, channel_multiplier=-1)
                    # out^T (+ softmax sums in row 64) accumulation
                    pout = psout.tile([P, QW], F32, tag="po")
                    for j in range(nj):
                        nc.tensor.matmul(
                            pout[:66, :], lhsT=_r(Vaug[:, j, h * 66:(h + 1) * 66]),
                            rhs=_r(atts[j]), start=(j == 0), stop=(j == nj - 1))
                    outT = outtp.tile([66, QW], F32R, tag="outT")
                    nc.scalar.copy(out=outT, in_=pout[:66, :])
                    recs = []
                    for qq in range(4):
                        g = 4 * i + qq
                        pt = psstg.tile([P, 512], F32, tag="stg")
                        nc.tensor.transpose(_r(pt[:, :66]),
                                            _r(outT[:, qq * P:(qq + 1) * P]),
                                            _r(ident[:66, :66]))
                        rec = recp.tile([P, 1], F32, tag="rec")
                        nc.vector.reciprocal(rec, pt[:, 64:65])
                        nc.vector.tensor_scalar_mul(
                            outnat[:, g, h * HD:(h + 1) * HD],
                            pt[:, :HD], rec)
                        recs.append(rec)
                    # attn rows: transpose attnT blocks, normalize on evict
                    for qq in range(4):
                        g = 4 * i + qq
                        row = rowp.tile([P, S], F32, tag="row")
                        for j0 in range(0, g + 1, 4):
                            jc = min(4, g + 1 - j0)
                            pst = psstg.tile([P, 512], F32, tag="stg")
                            for jj in range(jc):
                                nc.tensor.transpose(
                                    _r(pst[:, jj * P:(jj + 1) * P]),
                                    _r(atts[j0 + jj][:, qq * P:(qq + 1) * P]),
                                    _r(ident))
                            nc.vector.tensor_scalar_mul(
                                row[:, j0 * P:(j0 + jc) * P],
                                pst[:, :jc * P], recs[qq])
                        nc.sync.dma_start(
                            out=attnp[h, g * P:(g + 1) * P, :(g + 1) * P],
                            in_=row[:, :(g + 1) * P])
                # per-head slice of the final out DMA (shrinks the tail)
                nc.sync.dma_start(
                    out=outp[:, h * HD:(h + 1) * HD].rearrange(
                        "(g p) n -> p g n", p=P),
                    in_=outnat[:, :, h * HD:(h + 1) * HD])


    nc.compile()
    return nc


def _get_nc():
    if "nc" not in _CACHED:
        _CACHED["nc"] = _build_program()
    return _CACHED["nc"]


def _run(x, Wq, bq, Wk, bk, Wv, bv, trace=False):
    nc = _get_nc()
    x = np.ascontiguousarray(x, dtype=np.float32)
    in_maps = []
    for c in range(N_CORES):
        b, hg = divmod(c, N_CORES // B)
        cs = slice(hg * DHC, (hg + 1) * DHC)
        in_maps.append({
            "xin": x[b],
            "wq": np.ascontiguousarray(Wq[:, cs], dtype=np.float32),
            "wk": np.ascontiguousarray(Wk[:, cs], dtype=np.float32),
            "wv": np.ascontiguousarray(Wv[:, cs], dtype=np.float32),
            "bq": np.ascontiguousarray(bq[cs], dtype=np.float32),
            "bk": np.ascontiguousarray(bk[cs], dtype=np.float32),
            "bv": np.ascontiguousarray(bv[cs], dtype=np.float32),
        })
    res = run_bass_kernel_spmd(nc, in_maps, list(range(N_CORES)),
                               trace=trace)
    out = np.empty((B, S, D), np.float32)
    attn = np.empty((B, H, S, S), np.float32)
    K = np.empty((B, S, D), np.float32)
    V = np.empty((B, S, D), np.float32)
    for c in range(N_CORES):
        b, hg = divmod(c, N_CORES // B)
        cs = slice(hg * DHC, (hg + 1) * DHC)
        r = res.results[c]
        out[b][:, cs] = r["outp"]
        attn[b, hg * NHC:(hg + 1) * NHC] = r["attnp"]
        K[b][:, cs] = r["kp"]
        V[b][:, cs] = r["vp"]
    return (out, attn, K, V), res


def kernel(x, Wq, bq, Wk, bk, Wv, bv):
    outs, _ = _run(x, Wq, bq, Wk, bk, Wv, bv, trace=False)
    return outs
